# revision 17
# baseline (speedup 1.0000x reference)
"""CenterLoss kernel for Trainium2 (Bass/Tile), data-parallel over 8 NeuronCores.

loss = 0.5 * sum_i ||x_i - centers[targets_i]||^2

The reference materializes the full [N, C] distance matrix and gathers one
entry per row; here we gather only the target center rows and fuse the
subtract into the gather DMA itself where possible.

Sharding: inputs/targets split along batch N across 8 cores (512 rows each),
centers replicated. Each core PE-reduces its per-partition partials to a
[1, 8] row and ships 32 bytes; the host sums across cores and scales by 0.5.

Design notes (all measured on HW traces):
  - The gather uses gpsimd `indirect_dma_start` (resident SWDGE IndirectCopy,
    one row per partition per 128-row chunk) rather than `dma_gather`, whose
    loadable `mlp` ucode library costs ~11 us of IRAM load before the first
    descriptor.
  - centers live in HBM as fp8 e4m3 (2e-2 rel-err budget; quantization error
    ~4e-4) and the SWDGE DMA upcasts to bf16 in flight, so SBUF compute keeps
    the DVE 2x mode (cayman DVE has no fp8 packing).
  - Each chunk: DVE adds -x (host ships x negated), then the square+row-sum
    is split 896/128 between ACT (fused square+accumulate) and DVE
    (mult+reduce) so neither engine is the tail.
  - idx rides the SP HWDGE ring (SDMA queue row 1) and x the ACT ring (row
    10): queue rows drain in strict priority, so the 2 KB idx transfer is
    never starved behind the x stream.
"""

import numpy as np
import ml_dtypes

import concourse.bacc as bacc
import concourse.bass as bass
import concourse.tile as tile
from concourse import mybir
from concourse.bass_utils import run_bass_kernel_spmd

N, C, D = 4096, 8192, 1024
N_CORES = 8
ROWS = N // N_CORES  # 512 rows per core
P = 128              # SBUF partitions
CHUNKS = ROWS // P   # 4 chunks of 128 rows
NACC = 2 * CHUNKS    # per chunk: one ACT accum col + one DVE reduce col
FA = 832             # cols squared on ACT per chunk (rest: DVE mult+reduce)
FA_LAST = 512        # last chunk leans harder on DVE to shorten the tail

BF16 = mybir.dt.bfloat16

# Stashed BassKernelResults from the most recent kernel() call (for profiling).
LAST_RESULTS = None
_NC_CACHE = None


def _build_bass():
    nc = bacc.Bacc("TRN2", target_bir_lowering=False)
    x = nc.dram_tensor("x", [P, CHUNKS * D], BF16, kind="ExternalInput")
    idx = nc.dram_tensor("idx", [P, CHUNKS], mybir.dt.int32, kind="ExternalInput")
    centers = nc.dram_tensor("centers", [C, D], mybir.dt.float8e4, kind="ExternalInput")
    out = nc.dram_tensor("out", [1, NACC], mybir.dt.float32, kind="ExternalOutput")

    with tile.TileContext(nc) as tc:
        with (
            tc.tile_pool(name="io", bufs=1) as io,
            tc.tile_pool(name="cpool", bufs=CHUNKS) as cp,
            tc.tile_pool(name="psum", bufs=1, space="PSUM") as pp,
            tc.tile_pool(name="small", bufs=1) as small,
        ):
            # idx first on the SP ring (HWDGE; measured ~2.1 us issue->sem,
            # faster than the pool SWDGE route whose Q7 emission queues
            # behind framework preamble work).
            idx_sb = small.tile([P, CHUNKS], mybir.dt.int32)
            nc.sync.dma_start(idx_sb[:], idx[:, :])

            ones = small.tile([P, 1], mybir.dt.float32)
            nc.vector.memset(ones[:], 1.0)
            # Dummy activation to pull the ACT function-table load off the
            # critical path.
            warm = small.tile([1, 1], mybir.dt.float32)
            nc.scalar.activation(
                out=warm[:], in_=ones[0:1, :],
                func=mybir.ActivationFunctionType.Square,
            )

            # -x, pre-swizzled: column block t of partition p holds shard row
            # t*128 + p. One DMA per chunk so the CCE gathers gate only on
            # their own chunk.
            # Alternate the chunk DMAs across both HWDGE rings (SP ring
            # behind idx, ACT ring otherwise free) so the 1 MB x stream ramps
            # to full bandwidth before the gather transfers need quiet HBM.
            x_sb = io.tile([P, CHUNKS * D], BF16, tag="x")
            for t in range(CHUNKS):
                eng = nc.scalar if t % 2 == 0 else nc.sync
                eng.dma_start(
                    x_sb[:, t * D : (t + 1) * D], x[:, t * D : (t + 1) * D]
                )

            acc = small.tile([P, NACC], mybir.dt.float32)
            # Four 128-row gathers (one row per partition per op).
            ct = []
            for t in range(CHUNKS):
                cg = cp.tile([P, D], BF16, tag=f"c{t}")
                nc.gpsimd.indirect_dma_start(
                    out=cg[:],
                    out_offset=None,
                    in_=centers[:, :],
                    in_offset=bass.IndirectOffsetOnAxis(
                        ap=idx_sb[:, t : t + 1], axis=0
                    ),
                )
                ct.append(cg)
            for t in range(CHUNKS):
                fa = FA_LAST if t == CHUNKS - 1 else FA
                dv = ct[t][:]
                xv = x_sb[:, t * D : (t + 1) * D]
                # d = c + (-x)
                nc.vector.tensor_add(dv, dv, xv)
                # acc col 2t = sum_{d<fa} d^2 (ACT fused square+row-sum)
                nc.scalar.activation(
                    out=dv[:, 0:fa],
                    in_=dv[:, 0:fa],
                    func=mybir.ActivationFunctionType.Square,
                    accum_out=acc[:, 2 * t : 2 * t + 1],
                )
                # acc col 2t+1 = sum_{d>=fa} d^2 (DVE mult+reduce)
                h1 = dv[:, fa:D]
                nc.vector.tensor_tensor(h1, h1, h1, op=mybir.AluOpType.mult)
                nc.vector.tensor_reduce(
                    acc[:, 2 * t + 1 : 2 * t + 2],
                    h1,
                    axis=mybir.AxisListType.X,
                    op=mybir.AluOpType.add,
                )
            # Partition-reduce on the (otherwise idle) PE: ones^T @ acc gives
            # [1, NACC]; one 32-byte, single-descriptor DMA ships it. (A
            # [128, NACC] store costs ~2 us more in small-descriptor drain
            # and write receipts.)
            psum = pp.tile([1, NACC], mybir.dt.float32, tag="ps")
            nc.tensor.matmul(
                psum[:], lhsT=ones[:], rhs=acc[:, :], start=True, stop=True
            )
            res = small.tile([1, NACC], mybir.dt.float32)
            nc.vector.tensor_copy(res[:], psum[:])
            nc.sync.dma_start(out[:, :], res[:])
    nc.finalize()
    return nc


def _get_nc():
    global _NC_CACHE
    if _NC_CACHE is None:
        _NC_CACHE = _build_bass()
    return _NC_CACHE


def kernel(inputs, targets, centers):
    global LAST_RESULTS
    x = np.asarray(inputs, dtype=np.float32)
    tgt = np.asarray(targets).astype(np.int32)
    cen = np.ascontiguousarray(
        np.asarray(centers, dtype=np.float32).astype(ml_dtypes.float8_e4m3)
    )
    assert x.shape == (N, D) and cen.shape == (C, D) and tgt.shape == (N,)

    xneg = (-x).astype(ml_dtypes.bfloat16)
    nc = _get_nc()
    in_maps = []
    for c in range(N_CORES):
        xs = xneg[c * ROWS : (c + 1) * ROWS]
        # partition p, column block t <- shard row t*128 + p
        xw = np.ascontiguousarray(
            xs.reshape(CHUNKS, P, D).transpose(1, 0, 2).reshape(P, CHUNKS * D)
        )
        # idx[p, t] = target row for shard row t*128 + p
        ts = tgt[c * ROWS : (c + 1) * ROWS]
        idxw = np.ascontiguousarray(ts.reshape(CHUNKS, P).T)
        in_maps.append({"x": xw, "idx": idxw, "centers": cen})

    res = run_bass_kernel_spmd(nc, in_maps, core_ids=list(range(N_CORES)))
    LAST_RESULTS = res

    total = 0.0
    for r in res.results:
        total += float(r["out"].astype(np.float64).sum())
    return np.array(0.5 * total, dtype=np.float32)


# revision 18
# speedup vs baseline: 1.0544x; 1.0544x over previous
"""CenterLoss kernel for Trainium2 (Bass/Tile), data-parallel over 8 NeuronCores.

loss = 0.5 * sum_i ||x_i - centers[targets_i]||^2

The reference materializes the full [N, C] distance matrix and gathers one
entry per row; here we gather only the target center rows and fuse the
subtract into the gather DMA itself where possible.

Sharding: inputs/targets split along batch N across 8 cores (512 rows each),
centers replicated. Each core PE-reduces its per-partition partials to a
[1, 8] row and ships 32 bytes; the host sums across cores and scales by 0.5.

Design notes (all measured on HW traces):
  - The gather uses gpsimd `indirect_dma_start` (resident SWDGE IndirectCopy,
    one row per partition per 128-row chunk) rather than `dma_gather`, whose
    loadable `mlp` ucode library costs ~11 us of IRAM load before the first
    descriptor.
  - centers live in HBM as fp8 e4m3 (2e-2 rel-err budget; quantization error
    ~4e-4) and the SWDGE DMA upcasts to bf16 in flight, so SBUF compute keeps
    the DVE 2x mode (cayman DVE has no fp8 packing).
  - Each chunk: DVE adds -x (host ships x negated), then the square+row-sum
    is split 896/128 between ACT (fused square+accumulate) and DVE
    (mult+reduce) so neither engine is the tail.
  - idx rides the SP HWDGE ring (SDMA queue row 1) and x the ACT ring (row
    10): queue rows drain in strict priority, so the 2 KB idx transfer is
    never starved behind the x stream.
"""

import numpy as np
import ml_dtypes

import concourse.bacc as bacc
import concourse.bass as bass
import concourse.tile as tile
from concourse import mybir
from concourse.bass_utils import run_bass_kernel_spmd

N, C, D = 4096, 8192, 1024
N_CORES = 8
ROWS = N // N_CORES  # 512 rows per core
P = 128              # SBUF partitions
CHUNKS = ROWS // P   # 4 chunks of 128 rows
NACC = 2 * CHUNKS    # per chunk: one ACT accum col + one DVE reduce col
FA = 832             # cols squared on ACT per chunk (rest: DVE mult+reduce)
FA_LAST = 512        # last chunk leans harder on DVE to shorten the tail

BF16 = mybir.dt.bfloat16

# Stashed BassKernelResults from the most recent kernel() call (for profiling).
LAST_RESULTS = None
_NC_CACHE = None


def _build_bass():
    nc = bacc.Bacc("TRN2", target_bir_lowering=False)
    x = nc.dram_tensor("x", [P, CHUNKS * D], mybir.dt.float8e4, kind="ExternalInput")
    idx = nc.dram_tensor("idx", [P, CHUNKS], mybir.dt.int32, kind="ExternalInput")
    centers = nc.dram_tensor("centers", [C, D], mybir.dt.float8e4, kind="ExternalInput")
    out = nc.dram_tensor("out", [1, NACC], mybir.dt.float32, kind="ExternalOutput")

    with tile.TileContext(nc) as tc:
        with (
            tc.tile_pool(name="io", bufs=1) as io,
            tc.tile_pool(name="cpool", bufs=CHUNKS) as cp,
            tc.tile_pool(name="psum", bufs=1, space="PSUM") as pp,
            tc.tile_pool(name="small", bufs=1) as small,
        ):
            # idx first on the SP ring (HWDGE; measured ~2.1 us issue->sem,
            # faster than the pool SWDGE route whose Q7 emission queues
            # behind framework preamble work).
            idx_sb = small.tile([P, CHUNKS], mybir.dt.int32)
            nc.sync.dma_start(idx_sb[:], idx[:, :])

            ones = small.tile([P, 1], mybir.dt.float32)
            nc.vector.memset(ones[:], 1.0)
            # Dummy activation to pull the ACT function-table load off the
            # critical path.
            warm = small.tile([1, 1], mybir.dt.float32)
            nc.scalar.activation(
                out=warm[:], in_=ones[0:1, :],
                func=mybir.ActivationFunctionType.Square,
            )

            # -x, pre-swizzled: column block t of partition p holds shard row
            # t*128 + p. One DMA per chunk so the CCE gathers gate only on
            # their own chunk.
            # x comes in as fp8 via ONE SWDGE cast-DMA (fp8 -> bf16 in
            # flight): the Q7 emits its descriptors at body start while the
            # pool queue is otherwise idle, the 0.5 MB stream clears HBM
            # ~3 us before the gather reads start, and SBUF stays bf16 for
            # the DVE 2x mode.
            x_sb = io.tile([P, CHUNKS * D], BF16, tag="x")
            nc.gpsimd.dma_start(x_sb[:], x[:, :])

            acc = small.tile([P, NACC], mybir.dt.float32)
            # Four 128-row gathers (one row per partition per op).
            ct = []
            for t in range(CHUNKS):
                cg = cp.tile([P, D], BF16, tag=f"c{t}")
                nc.gpsimd.indirect_dma_start(
                    out=cg[:],
                    out_offset=None,
                    in_=centers[:, :],
                    in_offset=bass.IndirectOffsetOnAxis(
                        ap=idx_sb[:, t : t + 1], axis=0
                    ),
                )
                ct.append(cg)
            for t in range(CHUNKS):
                fa = FA_LAST if t == CHUNKS - 1 else FA
                dv = ct[t][:]
                xv = x_sb[:, t * D : (t + 1) * D]
                # d = c + (-x)
                nc.vector.tensor_add(dv, dv, xv)
                # acc col 2t = sum_{d<fa} d^2 (ACT fused square+row-sum)
                nc.scalar.activation(
                    out=dv[:, 0:fa],
                    in_=dv[:, 0:fa],
                    func=mybir.ActivationFunctionType.Square,
                    accum_out=acc[:, 2 * t : 2 * t + 1],
                )
                # acc col 2t+1 = sum_{d>=fa} d^2 (DVE mult+reduce)
                h1 = dv[:, fa:D]
                nc.vector.tensor_tensor(h1, h1, h1, op=mybir.AluOpType.mult)
                nc.vector.tensor_reduce(
                    acc[:, 2 * t + 1 : 2 * t + 2],
                    h1,
                    axis=mybir.AxisListType.X,
                    op=mybir.AluOpType.add,
                )
            # Partition-reduce on the (otherwise idle) PE: ones^T @ acc gives
            # [1, NACC]; one 32-byte, single-descriptor DMA ships it. (A
            # [128, NACC] store costs ~2 us more in small-descriptor drain
            # and write receipts.)
            psum = pp.tile([1, NACC], mybir.dt.float32, tag="ps")
            nc.tensor.matmul(
                psum[:], lhsT=ones[:], rhs=acc[:, :], start=True, stop=True
            )
            res = small.tile([1, NACC], mybir.dt.float32)
            nc.vector.tensor_copy(res[:], psum[:])
            nc.sync.dma_start(out[:, :], res[:])
    nc.finalize()
    return nc


def _get_nc():
    global _NC_CACHE
    if _NC_CACHE is None:
        _NC_CACHE = _build_bass()
    return _NC_CACHE


def kernel(inputs, targets, centers):
    global LAST_RESULTS
    x = np.asarray(inputs, dtype=np.float32)
    tgt = np.asarray(targets).astype(np.int32)
    cen = np.ascontiguousarray(
        np.asarray(centers, dtype=np.float32).astype(ml_dtypes.float8_e4m3)
    )
    assert x.shape == (N, D) and cen.shape == (C, D) and tgt.shape == (N,)

    xneg = (-x).astype(ml_dtypes.float8_e4m3)
    nc = _get_nc()
    in_maps = []
    for c in range(N_CORES):
        xs = xneg[c * ROWS : (c + 1) * ROWS]
        # partition p, column block t <- shard row t*128 + p
        xw = np.ascontiguousarray(
            xs.reshape(CHUNKS, P, D).transpose(1, 0, 2).reshape(P, CHUNKS * D)
        )
        # idx[p, t] = target row for shard row t*128 + p
        ts = tgt[c * ROWS : (c + 1) * ROWS]
        idxw = np.ascontiguousarray(ts.reshape(CHUNKS, P).T)
        in_maps.append({"x": xw, "idx": idxw, "centers": cen})

    res = run_bass_kernel_spmd(nc, in_maps, core_ids=list(range(N_CORES)))
    LAST_RESULTS = res

    total = 0.0
    for r in res.results:
        total += float(r["out"].astype(np.float64).sum())
    return np.array(0.5 * total, dtype=np.float32)
